# revision 48
# baseline (speedup 1.0000x reference)
"""GQA attention + RoPE + O-proj, tensor-parallel over 8 NeuronCores.

Strategy (head-parallel TP + all-to-all reshard before O-proj), bf16
matmul inputs with fp32 PSUM accumulation throughout:
  - host: transpose x -> xT [DIM, T] bf16; shuffle per-head wq/wk columns
    to [even hd | odd hd] so RoPE works in the transposed layout.
  - core c: projects q for heads {2c, 2c+1} and k,v for kv-head c//2 over
    all tokens (weight-stationary bf16 matmuls, xT streamed in quarter
    tiles with first-needed-first DMA issue), applies RoPE inline per
    token-pair (sign-folded), transposes V inline (PE transpose, scalar-
    engine drain), then runs causal attention in S^T [k, q] layout one
    512-wide q-group at a time with no-max softmax (scores ~N(0,1)):
    scores -> exp on ACT -> bf16 P strips; softmax denominators accumulate
    in a PSUM bank via ones-matmuls (pairs of full-width strips pre-summed
    on DVE to halve the matmul count); P@V accumulates O^T per group; fast
    Newton reciprocal normalizes.
  - Two AllToAlls (one per local head) reshard attention outputs
    head-major -> token-sharded; the first overlaps the second head's
    attention, the second overlaps the head-0 half of the O-projection
    (computed into bf16 SBUF partials with all wo quarters preloaded
    during attention); the head-1 half then finishes and adds partials.
"""

import os
import numpy as np
import ml_dtypes

import concourse.bass as bass
import concourse.bacc as bacc
import concourse.tile as tile
from concourse import mybir
from concourse.bass_utils import run_bass_kernel_spmd

F32 = mybir.dt.float32
F32R = mybir.dt.float32r
BF16 = mybir.dt.bfloat16

N_CORES = 8

# Full-problem config (hardcoded per spec).
B, SB, DIM = 2, 2048, 2048         # batches, seq per batch, model dim
H, HKV, HD = 16, 4, 128            # q heads, kv heads, head dim
SCALE = 1.0 / float(np.sqrt(HD))

T = B * SB                          # 4096 flat tokens (batch-major)
TPC = T // N_CORES                  # 512 tokens per core (output shard)
HPC = H // N_CORES                  # 2 q heads per core
QW = HPC * HD                       # 256 q cols per core
NKD = DIM // 128                    # 16 contraction tiles for projections
NG = SB // 512                      # 4 q-groups of 512 per batch
KT = SB // 128                      # 16 k-tiles per batch
NTT = T // 128                      # 32 token tiles total
NHD = (H * HD) // 128               # 16 hd row-tiles of wo


def _build():
    nc = bacc.Bacc("TRN2", target_bir_lowering=False, debug=False,
                   num_devices=N_CORES)

    xT = nc.dram_tensor("xT", [DIM, T], BF16, kind="ExternalInput").ap()
    wq_c = nc.dram_tensor("wq_c", [DIM, QW], BF16, kind="ExternalInput").ap()
    wk_c = nc.dram_tensor("wk_c", [DIM, HD], BF16, kind="ExternalInput").ap()
    wv_c = nc.dram_tensor("wv_c", [DIM, HD], BF16, kind="ExternalInput").ap()
    wo_f = nc.dram_tensor("wo_f", [H * HD, DIM], BF16, kind="ExternalInput").ap()
    cosd = nc.dram_tensor("cosd", [128, SB], BF16, kind="ExternalInput").ap()
    sind = nc.dram_tensor("sind", [128, SB], BF16, kind="ExternalInput").ap()
    sgn = nc.dram_tensor("sgn", [128, 1], F32, kind="ExternalInput").ap()
    tri = nc.dram_tensor("tri", [128, 512], BF16, kind="ExternalInput").ap()
    ones = nc.dram_tensor("ones", [128, 128], BF16, kind="ExternalInput").ap()
    ident = nc.dram_tensor("ident", [128, 128], BF16, kind="ExternalInput").ap()
    out_c = nc.dram_tensor("out_c", [TPC, DIM], F32, kind="ExternalOutput").ap()

    a2a_in = []
    a2a_out = []
    for hl in range(HPC):
        a2a_in.append(nc.dram_tensor(f"a2a_in{hl}",
                                     [N_CORES, HD, TPC], BF16).ap())
        a2a_out.append(nc.dram_tensor(f"a2a_out{hl}",
                                      [N_CORES, HD, TPC], BF16).ap())

    SEG = min(1024, SB)            # rope segment (never crosses a batch)
    NKQ = max(1, NKD // 4)         # dim-tiles per xt quarter
    NQT = NKD // NKQ               # quarters per token group

    with tile.TileContext(nc) as tc:
        with tc.tile_pool(name="const", bufs=1) as constp, \
             tc.tile_pool(name="qkv", bufs=1) as qkvp:
            ident_sb = constp.tile([128, 128], BF16)
            nc.sync.dma_start(ident_sb[:], ident[:, :])
            sgn_sb = constp.tile([128, 1], F32)
            nc.sync.dma_start(sgn_sb[:], sgn[:, :])

            # persistent roped projections + V in natural layout
            qT0 = qkvp.tile([128, T], BF16, tag="qT0")
            qT1 = qkvp.tile([128, T], BF16, tag="qT1")
            kT = qkvp.tile([128, T], BF16, tag="kT")
            vT = qkvp.tile([128, T], BF16, tag="vT")
            chunks = [qT0, qT1, kT]

            # ------ phase 1: projections + inline RoPE + V transpose ------
            with tc.tile_pool(name="w", bufs=1) as wp, \
                 tc.tile_pool(name="cs", bufs=1) as csp, \
                 tc.tile_pool(name="xt", bufs=7) as xtp, \
                 tc.tile_pool(name="rtmp", bufs=1) as rp, \
                 tc.tile_pool(name="pproj", bufs=1, space="PSUM") as pp:
                wq_sb = wp.tile([128, NKD * QW], BF16)
                wk_sb = wp.tile([128, NKD * HD], BF16)
                wv_sb = wp.tile([128, NKD * HD], BF16)
                # split weight loads into kk-quarter chunks on separate DMA
                # queues so the kk=0 chunks (first matmul) land fast
                wq4 = wq_c.rearrange("(n p) m -> p n m", p=128)
                wk4 = wk_c.rearrange("(n p) m -> p n m", p=128)
                wv4 = wv_c.rearrange("(n p) m -> p n m", p=128)
                wq_v = wq_sb.rearrange("p (n m) -> p n m", n=NKD)
                wk_v = wk_sb.rearrange("p (n m) -> p n m", n=NKD)
                wv_v = wv_sb.rearrange("p (n m) -> p n m", n=NKD)
                xT3 = xT.rearrange("(n p) m -> p n m", p=128)  # [128,NKD,T]

                def load_xt(q, g, eng=None, h=None):
                    xt_q = xtp.tile([128, NKQ * 512], BF16, tag="xt",
                                    name=f"xt{q}_{g}")
                    xt_v = xt_q.rearrange("p (n m) -> p n m", n=NKQ)
                    if h is None:
                        h = NKQ // 2
                    for kq in range(0, NKQ, h):   # chunks: separate queues
                        (eng or nc.sync).dma_start(
                            xt_v[:, kq:kq + h],
                            xT3[:, q * NKQ + kq:q * NKQ + kq + h,
                                g * 512:(g + 1) * 512])
                    return xt_q

                # first-needed DMAs first: kk0-3 weight chunks, then pair-0
                # q=0 x tiles at kk granularity so the kk=0 chunk (the very
                # first matmul's moving operand) lands fastest.
                pre0 = [load_xt(0, 0, nc.scalar, h=1),
                        load_xt(0, 1, nc.scalar, h=1)]
                nc.sync.dma_start(wq_v[:, 0:4], wq4[:, 0:4])
                nc.sync.dma_start(wk_v[:, 0:4], wk4[:, 0:4])
                nc.sync.dma_start(wv_v[:, 0:4], wv4[:, 0:4])
                for k0 in range(4, NKD, 4):
                    nc.sync.dma_start(wq_v[:, k0:k0 + 4], wq4[:, k0:k0 + 4])
                    nc.sync.dma_start(wk_v[:, k0:k0 + 4], wk4[:, k0:k0 + 4])
                    nc.sync.dma_start(wv_v[:, k0:k0 + 4], wv4[:, k0:k0 + 4])
                def w_slice(c, kk):
                    if c < 2:
                        return wq_sb[:, kk * QW + c * 128: kk * QW + (c + 1) * 128]
                    if c == 2:
                        return wk_sb[:, kk * HD:(kk + 1) * HD]
                    return wv_sb[:, kk * HD:(kk + 1) * HD]

                npair = T // 1024
                cos_sb = sin_sb = None
                for p in range(npair):
                    # stream this pair's xT as quarter tiles (kk-major use)
                    xts = [[], []]          # [grp][quarter]
                    for q in range(NQT):
                        for j, g in enumerate((2 * p, 2 * p + 1)):
                            if p == 0 and q == 0:
                                xts[j].append(pre0[j])
                            else:
                                xts[j].append(load_xt(q, g))
                    if cos_sb is None:
                        cos_sb = csp.tile([128, SB], BF16)
                        nc.sync.dma_start(cos_sb[:], cosd[:, :])
                        sin_sb = csp.tile([128, SB], BF16)
                        nc.sync.dma_start(sin_sb[:], sind[:, :])
                    pss = []
                    for c in range(4):
                        ps_c = pp.tile([128, 1024], F32, tag=f"pp{c}")
                        pss.append(ps_c)
                    for kk in range(NKD):
                        for c in range(4):
                            lhsT = w_slice(c, kk)
                            for j in (0, 1):
                                nc.tensor.matmul(
                                    pss[c][:, j * 512:(j + 1) * 512], lhsT,
                                    xts[j][kk // NKQ][:, (kk % NKQ) * 512:
                                                      (kk % NKQ + 1) * 512],
                                    start=(kk == 0), stop=(kk == NKD - 1))
                    # drain q0/q1/k with RoPE staged below; v via transpose
                    cp0 = 1024 * p
                    for c in range(3):
                        nc.vector.tensor_copy(
                            chunks[c][:, cp0:cp0 + 1024], pss[c][:])
                    nc.vector.tensor_copy(vT[:, cp0:cp0 + 1024], pss[3][:])
                    # RoPE on the pair's columns, per batch segment
                    for s0 in range(cp0, cp0 + 1024, SEG):
                        pos0 = s0 % SB
                        for X in chunks:
                            tcs = rp.tile([128, SEG], BF16, tag="tc")
                            nc.vector.tensor_tensor(
                                tcs[:], X[:, s0:s0 + SEG],
                                cos_sb[:, pos0:pos0 + SEG],
                                op=mybir.AluOpType.mult)
                            tsn = rp.tile([128, SEG], BF16, tag="ts")
                            nc.vector.tensor_tensor(
                                tsn[:], X[:, s0:s0 + SEG],
                                sin_sb[:, pos0:pos0 + SEG],
                                op=mybir.AluOpType.mult)
                            tsw = rp.tile([128, SEG], BF16, tag="tw")
                            nc.sync.dma_start(tsw[0:64, :], tsn[64:128, :])
                            nc.sync.dma_start(tsw[64:128, :], tsn[0:64, :])
                            # X = tcs + sgn * tsw   (sgn = -1 top / +1 bottom)
                            nc.vector.scalar_tensor_tensor(
                                X[:, s0:s0 + SEG], tsw[:], sgn_sb[:, 0:1],
                                tcs[:], op0=mybir.AluOpType.mult,
                                op1=mybir.AluOpType.add)

            # ---------------- phase 3: attention ----------------------
            DQ = DIM // 4
            wo3 = wo_f.rearrange("(n p) m -> p n m", p=128)  # [128,NHD,DIM]
            wop = tc.alloc_tile_pool(name="wop", bufs=4)
            wo_half = []
            with tc.tile_pool(name="att", bufs=2) as ap, \
                 tc.tile_pool(name="attc", bufs=1) as apc, \
                 tc.tile_pool(name="pstr", bufs=5) as pstr, \
                 tc.tile_pool(name="psS", bufs=3, space="PSUM") as psS, \
                 tc.tile_pool(name="psD", bufs=2, space="PSUM") as psD, \
                 tc.tile_pool(name="psO", bufs=2, space="PSUM") as psO:
                wo_sb0 = wop.tile([128, NHD * DQ], BF16, tag="wo")
                nc.sync.dma_start(
                    wo_sb0.rearrange("p (n m) -> p n m", n=NHD),
                    wo3[:, :, 0:DQ])
                wo_half.append(wo_sb0)
                tri_sb = apc.tile([128, 512], BF16)
                nc.sync.dma_start(tri_sb[:], tri[:, :])
                ones_sb = apc.tile([128, 128], BF16)
                nc.sync.dma_start(ones_sb[:], ones[:, :])
                Vt = qkvp.tile([128, T], BF16, tag="Vt")
                for ttg in range(NTT):
                    psv = psS.tile([128, 128], BF16, tag="S")
                    nc.tensor.transpose(psv[:],
                                        vT[:, ttg * 128:(ttg + 1) * 128],
                                        ident_sb[:])
                    nc.scalar.copy(Vt[:, ttg * 128:(ttg + 1) * 128],
                                   psv[:])
                for hl in range(HPC):
                    qTh = qT0 if hl == 0 else qT1
                    for b in range(B):
                        qb = b * SB     # q-col base for this batch
                        # flattened (q-group, k-tile) work list; q-group at a
                        # time so denominators accumulate in a [128,512] PSUM
                        # bank via ones-matmul (PE) instead of a DVE chain.
                        work = [(g, t) for g in range(NG)
                                for t in range(4 * g + 4)]
                        pOs, psrs, Ps, qlos = {}, {}, {}, {}

                        def emit_scores(i, hl=hl, b=b, qb=qb, work=work,
                                        pOs=pOs, psrs=psrs, Ps=Ps, qlos=qlos,
                                        qTh=qTh):
                            g, t = work[i]
                            qlo = 128 * (t - 4 * g) if t >= 4 * g else 0
                            w = 512 - qlo
                            if t == 0:
                                pOs[g] = psO.tile([128, 512], F32, tag="O",
                                                  name=f"pO{hl}{b}{g}")
                                psrs[g] = psD.tile([128, 512], F32, tag="D",
                                                   name=f"psr{hl}{b}{g}")
                            S = psS.tile([128, 512], F32, tag="S")
                            nc.tensor.matmul(
                                S[:, 0:w],
                                kT[:, qb + 128 * t: qb + 128 * (t + 1)],
                                qTh[:, qb + 512 * g + qlo:
                                    qb + 512 * (g + 1)],
                                start=True, stop=True)
                            P = pstr.tile([128, 512], BF16, tag="P")
                            nc.scalar.activation(
                                P[:, 0:w], S[:, 0:w],
                                mybir.ActivationFunctionType.Exp, scale=SCALE)
                            if t >= 4 * g:     # diagonal tile: causal mask
                                nc.vector.tensor_tensor(
                                    P[:, 0:w], P[:, 0:w], tri_sb[:, 0:w],
                                    op=mybir.AluOpType.mult)
                            Ps[i], qlos[i] = P, qlo

                        def emit_accum(i, hl=hl, b=b, work=work, pOs=pOs,
                                       psrs=psrs, Ps=Ps, qlos=qlos):
                            g, t = work[i]
                            qlo = qlos.pop(i)
                            w = 512 - qlo
                            P = Ps.pop(i)
                            last = (t == 4 * g + 3)
                            # denominator: merge pairs of full-width strips on
                            # DVE so the ones-matmul runs half as often
                            nondiag = t < 4 * g
                            if nondiag and t % 2 == 0:
                                Ps[("ev", g)] = P     # defer to the odd tile
                            else:
                                if nondiag:           # odd: sum with previous
                                    Pe = Ps.pop(("ev", g))
                                    P2 = pstr.tile([128, 512], BF16,
                                                   tag="P2")
                                    nc.vector.tensor_tensor(
                                        P2[:], Pe[:], P[:],
                                        op=mybir.AluOpType.add)
                                    dP, dst = P2, (t == 1)
                                else:
                                    dP, dst = P, (t == 0)
                                nc.tensor.matmul(
                                    psrs[g][:, qlo:512], ones_sb[:],
                                    dP[:, 0:w],
                                    start=dst, stop=last,
                                    skip_group_check=True)
                            nc.tensor.matmul(
                                pOs[g][:, qlo:512],
                                Vt[:, (b * KT + t) * 128:
                                   (b * KT + t + 1) * 128],
                                P[:, 0:w],
                                start=(t == 0), stop=last,
                                skip_group_check=True)
                            if last:   # group done: normalize + ship
                                rb = ap.tile([128, 512], F32, tag="rb")
                                scr = ap.tile([128, 512], F32, tag="scr")
                                nc.vector.reciprocal_approx_accurate(
                                    rb[:], psrs[g][:], scr[:])
                                Ofin = ap.tile([128, 512], BF16, tag="Of")
                                nc.vector.tensor_tensor(
                                    Ofin[:], pOs[g][:], rb[:],
                                    op=mybir.AluOpType.mult)
                                nc.sync.dma_start(
                                    a2a_in[hl][b * NG + g, :, :], Ofin[:])

                        for i in range(len(work)):
                            emit_scores(i)
                            if i > 0:
                                emit_accum(i - 1)
                        emit_accum(len(work) - 1)
                    # per-head collective, overlaps the next head's attention
                    nc.gpsimd.collective_compute(
                        "AllToAll", mybir.AluOpType.bypass,
                        replica_groups=[list(range(N_CORES))],
                        ins=[a2a_in[hl].opt()], outs=[a2a_out[hl].opt()])

        # ---------------- phase 5: O-projection ----------------------
            kks0 = list(range(0, NHD, HPC))      # head-0 hd tiles
            kks1 = list(range(1, NHD, HPC)) if HPC > 1 else []
            NQO = DIM // DQ
            NTO = TPC // 128
            with tc.tile_pool(name="oproj", bufs=1) as op, \
                 tc.tile_pool(name="opa", bufs=16) as opa, \
                 tc.tile_pool(name="ostg", bufs=4) as ostg, \
                 tc.tile_pool(name="psop", bufs=4, space="PSUM") as pso:
                wos = {0: wo_half[0]}

                def get_wo(q):
                    if q not in wos:
                        w = wop.tile([128, NHD * DQ], BF16, tag="wo",
                                     name=f"wo{q}")
                        wv_ = w.rearrange("p (n m) -> p n m", n=NHD)
                        for n0 in range(0, NHD, 4):   # chunks: spread queues
                            nc.sync.dma_start(
                                wv_[:, n0:n0 + 4],
                                wo3[:, n0:n0 + 4, q * DQ:(q + 1) * DQ])
                        wos[q] = w
                    return wos[q]

                for q in range(1, NQO):   # preload: flows during attention
                    get_wo(q)
                recv = {}
                for kk in kks0 + kks1:
                    rv = op.tile([128, TPC], BF16, tag=f"rv{kk}",
                                 name=f"rv{kk}")
                    nc.sync.dma_start(rv[:], a2a_out[kk % HPC][kk // HPC, :, :])
                    recv[kk] = rv

                # phase A: head-0 contributions only (needs just the first
                # AllToAll) -> bf16 partials in SBUF; overlaps the second.
                # kk-major inner order: two q-halves back to back share the
                # same stationary recv chunk.
                pA = {}
                for qp in range(NQO // 2):
                    qs = (2 * qp, 2 * qp + 1)
                    for q in qs:
                        get_wo(q)
                    for tt in range(NTO):
                        pos = {}
                        for q in qs:
                            pos[q] = pso.tile([128, DQ], F32, tag="po",
                                              name=f"poA{q}{tt}")
                        for ki, kk in enumerate(kks0):
                            lhs = recv[kk][:, tt * 128:(tt + 1) * 128]
                            for q in qs:
                                nc.tensor.matmul(
                                    pos[q][:], lhs,
                                    wos[q][:, kk * DQ:(kk + 1) * DQ],
                                    start=(ki == 0),
                                    stop=(ki == len(kks0) - 1),
                                    skip_group_check=True)
                        for q in qs:
                            pa = opa.tile([128, DQ], BF16, tag="pa",
                                          name=f"pa{q}{tt}")
                            nc.vector.tensor_copy(pa[:], pos[q][:])
                            pA[(q, tt)] = pa
                # phase B: head-1 contributions + combine + store
                for qp in range(NQO // 2):
                    qs = (2 * qp, 2 * qp + 1)
                    for tt in range(NTO):
                        pos = {}
                        for q in qs:
                            pos[q] = pso.tile([128, DQ], F32, tag="po",
                                              name=f"poB{q}{tt}")
                        for ki, kk in enumerate(kks1):
                            lhs = recv[kk][:, tt * 128:(tt + 1) * 128]
                            for q in qs:
                                nc.tensor.matmul(
                                    pos[q][:], lhs,
                                    wos[q][:, kk * DQ:(kk + 1) * DQ],
                                    start=(ki == 0),
                                    stop=(ki == len(kks1) - 1),
                                    skip_group_check=True)
                        for q in qs:
                            stg = ostg.tile([128, DQ], F32, tag="stg")
                            nc.vector.tensor_tensor(
                                stg[:], pos[q][:], pA[(q, tt)][:],
                                op=mybir.AluOpType.add)
                            for hf in range(2):   # halves: parallel queues
                                nc.sync.dma_start(
                                    out_c[tt * 128:(tt + 1) * 128,
                                          q * DQ + hf * (DQ // 2):
                                          q * DQ + (hf + 1) * (DQ // 2)],
                                    stg[:, hf * (DQ // 2):
                                        (hf + 1) * (DQ // 2)])
            wop.release()

    if not nc.is_finalized():
        nc.finalize()
    return nc


_NC_CACHE = {}


def _get_nc():
    if "nc" not in _NC_CACHE:
        _NC_CACHE["nc"] = _build()
    return _NC_CACHE["nc"]


def _prep_inputs(x, cos, sin, wq, wk, wv, wo):
    x = np.asarray(x, np.float32)
    cos = np.asarray(cos, np.float32)
    sin = np.asarray(sin, np.float32)
    wq = np.asarray(wq, np.float32)
    wk = np.asarray(wk, np.float32)
    wv = np.asarray(wv, np.float32)
    wo = np.asarray(wo, np.float32)

    bf16 = ml_dtypes.bfloat16
    xT = np.ascontiguousarray(x.reshape(T, DIM).T).astype(bf16)
    perm = np.r_[np.arange(0, HD, 2), np.arange(1, HD, 2)]
    wq_sh = wq.reshape(DIM, H, HD)[:, :, perm].astype(bf16)
    wk_sh = wk.reshape(DIM, HKV, HD)[:, :, perm].astype(bf16)
    wv_r = wv.reshape(DIM, HKV, HD).astype(bf16)
    cosT = np.ascontiguousarray(cos.T)          # [64, SB]
    cosd_a = np.vstack([cosT, cosT]).astype(bf16)   # [128, SB]
    sinT = np.ascontiguousarray(sin.T)
    sind_a = np.vstack([sinT, sinT]).astype(bf16)
    sgn_a = np.vstack([np.full((64, 1), -1.0, np.float32),
                       np.full((64, 1), 1.0, np.float32)])
    tri_a = (np.arange(512)[None, :] >= np.arange(128)[:, None]
             ).astype(bf16)
    ones_a = np.ones((128, 128), bf16)
    ident_a = np.eye(128, dtype=bf16)
    wo_b = wo.astype(bf16)

    in_maps = []
    for c in range(N_CORES):
        h0 = HPC * c
        g = h0 // (H // HKV)
        in_maps.append({
            "xT": xT,
            "wq_c": np.ascontiguousarray(
                wq_sh[:, h0:h0 + HPC].reshape(DIM, QW)),
            "wk_c": np.ascontiguousarray(wk_sh[:, g]),
            "wv_c": np.ascontiguousarray(wv_r[:, g]),
            "wo_f": wo_b,
            "cosd": cosd_a, "sind": sind_a, "sgn": sgn_a, "tri": tri_a,
            "ones": ones_a, "ident": ident_a,
        })
    return in_maps


def _run(inputs, trace=False):
    in_maps = _prep_inputs(**inputs)
    nc = _get_nc()
    res = run_bass_kernel_spmd(
        nc, in_maps, core_ids=list(range(N_CORES)), trace=trace,
        trace_cores=list(range(N_CORES)) if trace else None)
    out = np.concatenate([res.results[c]["out_c"] for c in range(N_CORES)],
                         axis=0)
    return out.reshape(B, SB, DIM), res


def kernel(**inputs):
    out, _ = _run(inputs, trace=os.environ.get("KERNEL_TRACE", "0") == "1")
    return out

